# revision 17
# baseline (speedup 1.0000x reference)
"""Trainium2 Bass kernel for nn_MultiHeadAttention_77154792505388.

8-core sharding: data-parallel over batch (N=2) x tensor-parallel over heads
(16 heads -> 4 per core). Each core computes LN + projections + RoPE + grok
attention + o_proj for its (batch, head-group) and returns a partial o_proj
output; the host sums the 4 head-group partials per batch.

Device-side design (per core):
  - LayerNorm folded into the projection matmuls: stats (mean/var) via
    bn_stats on natural-layout X tiles; X^T produced by PE transposes; the
    projection computes  psum = X^T.T @ (g.W)  +  mu (x) (-colsum(gW))  and
    the PSUM evacuation applies the 1/sigma row scale (and RoPE for q/k).
  - RoPE in de-interleaved head layout (even dims then odd dims) so the
    pair-swap is a contiguous-block swap; the permutation is folded into the
    columns of Wq/Wk on the host (scores are invariant to it).
  - Attention computed transposed: S^T[tk, tq] tiles (contraction over
    d_head=64, two heads row-packed in the PE array), grok-tanh + exp on the
    scalar engine (single act table set), causal handled by variable matmul
    windows + an additive diagonal-block mask, softmax denominators via an
    appended ones-column in V (no extra reduction pass), division folded
    into the PSUM evacuation of the attention output.
  - o_proj with the attention output already in transposed (lhsT) layout.
"""

from contextlib import ExitStack

import numpy as np

import tile_patch

tile_patch.apply_patch()

import concourse.bass as bass
import concourse.mybir as mybir
import concourse.tile as tile
from concourse.bass_utils import run_bass_kernel_spmd
from concourse.masks import make_identity

F32 = mybir.dt.float32
F32R = mybir.dt.float32r
U32 = mybir.dt.uint32
ALU = mybir.AluOpType
AF = mybir.ActivationFunctionType

D_MODEL = 1024
N_HEADS = 16
D_HEAD = 64
T = 2048
EPS = 1e-5
GROK = 30.0
ROPE_BASE = 10000.0
N_CORES = 8
H_LOC = 4            # heads per core
DH = H_LOC * D_HEAD  # 256 local head dims
NEG = -1.0e30

TC = T // 128        # 16 t-chunks
DC = D_MODEL // 128  # 8 d-chunks
NJ = T // 512        # 4 tq tiles

# --- tuning flags ---
USE_TANH = True       # exact grok tanh (2 ACT passes) vs plain exp (1 pass)
F32R_MM = True        # float32r matmuls (full-rate, FP22 mantissa)
F32R_TRANS = False    # float32r PE transposes


def _mm(ap):
    return ap.bitcast(F32R) if F32R_MM else ap


def _tr(ap):
    return ap.bitcast(F32R) if F32R_TRANS else ap


class KB:
    """Kernel builder."""

    def __init__(self):
        self.nc = bass.Bass("TRN2", target_bir_lowering=False, debug=False,
                            num_devices=N_CORES)
        self.qh_tiles = {}

    def build(self):
        nc = self.nc
        d = {}
        for name, shape in [
            ("xq", [T, D_MODEL]), ("xk", [T, D_MODEL]), ("xv", [T, D_MODEL]),
            ("wq", [D_MODEL, DH]), ("wk", [D_MODEL, DH]), ("wv", [D_MODEL, DH]),
            ("wo", [DH, D_MODEL]),
            ("cosA", [T, DH]), ("sinA", [T, DH]),
            ("negcq_q", [1, DH]), ("negcq_k", [1, DH]), ("negcq_v", [1, DH]),
            ("padbias", [T]), ("maskT", [128, 128]),
            ("vconst", [128, TC * H_LOC * 36]),
        ]:
            d[name] = nc.dram_tensor(name, shape, F32, kind="ExternalInput").ap()
        self.d = d
        self.out_dram = nc.dram_tensor("out", [T, D_MODEL], F32,
                                       kind="ExternalOutput").ap()
        self.m_scratch = {s: nc.dram_tensor(f"mscr_{s}", [128, TC], F32).ap()
                          for s in "qkv"}
        self.rb_dr = nc.dram_tensor("rb_dr", [H_LOC, T], F32).ap()

        with tile.TileContext(nc) as tc:
            self.tc = tc
            with ExitStack() as ctx:
                self.emit(ctx)
        tile_patch.split_waits(nc)
        return nc

    def emit(self, ctx):
        nc, tc = self.nc, self.tc

        consts = ctx.enter_context(tc.tile_pool(name="consts", bufs=1))
        self.ident = consts.tile([128, 128], F32)
        make_identity(nc, self.ident)
        maskT_sb = consts.tile([128, 128], F32)
        nc.sync.dma_start(out=maskT_sb, in_=self.d["maskT"])
        padb_sb = consts.tile([128, TC], F32)
        nc.sync.dma_start(
            out=padb_sb, in_=self.d["padbias"].rearrange("(i p) -> p i", p=128))
        self.rsqrt_c = consts.tile([128, TC], U32)
        nc.vector.memset(self.rsqrt_c, 0x5F3759DF)

        big = ctx.enter_context(tc.tile_pool(name="big", bufs=1))
        self.qhT = [big.tile([128, T], F32, tag=f"qhT{p}", name=f"qhT{p}")
                    for p in range(2)]
        self.khT = [big.tile([128, T], F32, tag=f"khT{p}", name=f"khT{p}")
                    for p in range(2)]
        self.vh = big.tile([128, TC, H_LOC * 100], F32, tag="vh")
        vh4i = self.vh.rearrange("p i (h e) -> p i h e", h=H_LOC)
        nc.sync.dma_start(
            out=_mm(vh4i[:, :, :, D_HEAD:100]),
            in_=_mm(self.d["vconst"].rearrange(
                "p (i h c) -> p i h c", i=TC, h=H_LOC)))
        # newton-reciprocal seed constant
        self.recip_c = big.tile([128, 1024], mybir.dt.uint32, tag="recip_c")
        nc.vector.memset(self.recip_c, 0x7EF127EA)
        self.outT = [big.tile([128, T], F32, tag=f"outT{p}", name=f"outT{p}")
                     for p in range(2)]

        # ---- phase A ----
        with ExitStack() as actx:
            pools = dict(
                xt=actx.enter_context(tc.tile_pool(name="xt", bufs=1)),
                x=actx.enter_context(tc.tile_pool(name="xin", bufs=5)),
                st=actx.enter_context(tc.tile_pool(name="stats", bufs=2)),
                w=actx.enter_context(tc.tile_pool(name="wts", bufs=1)),
                small=actx.enter_context(tc.tile_pool(name="small", bufs=1)),
                rope=actx.enter_context(tc.tile_pool(name="rope", bufs=2)),
                qh=actx.enter_context(tc.tile_pool(name="qh", bufs=5)),
                tp_ps=actx.enter_context(
                    tc.tile_pool(name="tp_ps", bufs=2, space="PSUM")),
                pj_ps=actx.enter_context(
                    tc.tile_pool(name="pj_ps", bufs=2, space="PSUM")),
                qt_ps=actx.enter_context(
                    tc.tile_pool(name="qt_ps", bufs=2, space="PSUM")),
            )
            self.stream(pools, "q", self.d["xq"], self.d["wq"],
                        self.d["negcq_q"], rope=True, dests=self.qhT)
            self.stream(pools, "k", self.d["xk"], self.d["wk"],
                        self.d["negcq_k"], rope=True, dests=self.khT)
            self.stream(pools, "v", self.d["xv"], self.d["wv"],
                        self.d["negcq_v"], rope=False, dests=None)

        # ---- phase B ----
        with ExitStack() as bctx:
            st_ps = bctx.enter_context(
                tc.tile_pool(name="st_ps", bufs=3, space="PSUM"))
            ac_ps = bctx.enter_context(
                tc.tile_pool(name="ac_ps", bufs=1, space="PSUM"))
            pt_pool = bctx.enter_context(tc.tile_pool(name="pt", bufs=4))
            rs_pool = bctx.enter_context(tc.tile_pool(name="rs", bufs=2))
            rb_pool = bctx.enter_context(tc.tile_pool(name="rb", bufs=4))
            tb_pool = bctx.enter_context(tc.tile_pool(name="tb", bufs=2))
            self.attention(st_ps, ac_ps, pt_pool, rs_pool, rb_pool, tb_pool,
                           maskT_sb, padb_sb)

        # ---- phase C ----
        with ExitStack() as cctx:
            wo_pool = cctx.enter_context(tc.tile_pool(name="wo", bufs=1))
            of_pool = cctx.enter_context(tc.tile_pool(name="of", bufs=3))
            op_ps = cctx.enter_context(
                tc.tile_pool(name="op_ps", bufs=4, space="PSUM"))
            self.oproj(wo_pool, of_pool, op_ps)

    # ------------- stream: LN + transpose + proj (+rope) -------------

    def stream(self, pools, sname, x_dram, w_dram, negcq_dram, rope, dests):
        nc = self.nc
        w_sb = pools["w"].tile([128, DC, DH], F32, tag="w_sb")
        nc.sync.dma_start(
            out=_mm(w_sb),
            in_=_mm(w_dram.rearrange("(c p) n -> p c n", p=128)))
        negcq_sb = pools["small"].tile([1, DH], F32, tag="negcq")
        nc.sync.dma_start(out=_mm(negcq_sb), in_=_mm(negcq_dram))

        st = pools["st"]
        mv_all = st.tile([128, TC, 2], F32, tag="mv_all")
        xt = [pools["xt"].tile([128, T], F32, tag=f"xt{j}", name=f"xt{j}")
              for j in range(DC)]

        x_tiles = {}
        for g in range(TC // 4):
            for ii in range(4):
                i = 4 * g + ii
                x_in = pools["x"].tile([128, D_MODEL], F32, tag="x_in")
                nc.sync.dma_start(out=x_in, in_=x_dram[128 * i:128 * (i + 1), :])
                x_tiles[i] = x_in
                stats = st.tile([128, 2, 6], F32, tag="bn_st")
                xr = x_in.rearrange("p (s f) -> p s f", s=2)
                nc.vector.bn_stats(out=stats[:, 0, :], in_=xr[:, 0, :])
                nc.vector.bn_stats(out=stats[:, 1, :], in_=xr[:, 1, :])
                nc.vector.bn_aggr(out=mv_all[:, i, :], in_=stats)
            for j in range(DC):
                ps = pools["tp_ps"].tile([128, 512], F32, tag="tp")
                for ii in range(4):
                    i = 4 * g + ii
                    nc.tensor.transpose(
                        ps[:, 128 * ii:128 * (ii + 1)],
                        _tr(x_tiles[i][:, 128 * j:128 * (j + 1)]),
                        _tr(self.ident))
                nc.vector.tensor_copy(
                    out=_mm(xt[j][:, 512 * g:512 * (g + 1)]),
                    in_=ps.bitcast(F32R) if F32R_MM else ps)

        # batched rstd = rsqrt(var + eps), newton iteration from bit-trick seed
        w_t = st.tile([128, TC], F32, tag="nw_w")
        nc.vector.tensor_scalar_add(w_t, mv_all[:, :, 1], EPS)
        y = st.tile([128, TC], F32, tag="nw_y")
        tmp = st.tile([128, TC], F32, tag="nw_t")
        nc.vector.tensor_single_scalar(
            tmp.bitcast(U32), w_t.bitcast(U32), 1,
            op=ALU.logical_shift_right)
        nc.vector.tensor_tensor(
            y.bitcast(U32), self.rsqrt_c, tmp.bitcast(U32), op=ALU.subtract)
        a = st.tile([128, TC], F32, tag="nw_a")
        for _ in range(3):
            nc.vector.tensor_mul(a, y, y)
            nc.vector.tensor_mul(a, a, w_t)
            nc.vector.tensor_scalar(a, a, -0.5, 1.5,
                                    op0=ALU.mult, op1=ALU.add)
            nc.vector.tensor_mul(y, y, a)
        rstd = y

        # mean column-vector -> [1, T] row via a DRAM bounce
        mscr = self.m_scratch[sname]
        nc.sync.dma_start(out=mscr, in_=mv_all[:, :, 0])
        mu_flat = pools["small"].tile([1, T], F32, tag="mu_flat")
        nc.sync.dma_start(
            out=_mm(mu_flat.rearrange("o (i p) -> o i p", p=128)),
            in_=_mm(mscr.rearrange("p i -> i p")[None]))

        for i in range(TC):
            ps = pools["pj_ps"].tile([128, DH], F32, tag="pj")
            for j in range(DC):
                nc.tensor.matmul(
                    ps, _mm(xt[j][:, 128 * i:128 * (i + 1)]),
                    _mm(w_sb[:, j, :]), start=(j == 0), stop=False)
            nc.tensor.matmul(
                ps, _mm(mu_flat[:, 128 * i:128 * (i + 1)]), _mm(negcq_sb),
                start=False, stop=True)
            r_i = rstd[:, i:i + 1]
            if rope:
                cos_sb = pools["rope"].tile([128, DH], F32, tag="cos")
                sin_sb = pools["rope"].tile([128, DH], F32, tag="sin")
                nc.sync.dma_start(
                    out=cos_sb, in_=self.d["cosA"][128 * i:128 * (i + 1), :])
                nc.sync.dma_start(
                    out=sin_sb, in_=self.d["sinA"][128 * i:128 * (i + 1), :])
                sw = pools["rope"].tile([128, DH], F32, tag="sw")
                ps4 = ps.rearrange("p (h t e) -> p h t e", h=H_LOC, t=2)
                sw4 = sw.rearrange("p (h t e) -> p h t e", h=H_LOC, t=2)
                nc.vector.tensor_copy(out=sw4[:, :, 0, :], in_=ps4[:, :, 1, :])
                nc.vector.tensor_copy(out=sw4[:, :, 1, :], in_=ps4[:, :, 0, :])
                u1 = pools["qh"].tile([128, DH], F32, tag="u1")
                nc.vector.scalar_tensor_tensor(
                    out=u1, in0=ps, scalar=r_i, in1=cos_sb,
                    op0=ALU.mult, op1=ALU.mult)
                u2 = pools["rope"].tile([128, DH], F32, tag="u2")
                nc.vector.scalar_tensor_tensor(
                    out=u2, in0=sw, scalar=r_i, in1=sin_sb,
                    op0=ALU.mult, op1=ALU.mult)
                nc.vector.tensor_add(u1, u1, u2)
                self.qh_tiles.setdefault(sname, {})[i] = u1
                if i % 4 == 3:
                    g = i // 4
                    for p in range(2):
                        tps = pools["qt_ps"].tile([128, 512], F32, tag="qt")
                        for ii in range(4):
                            qh = self.qh_tiles[sname][4 * g + ii]
                            nc.tensor.transpose(
                                tps[:, 128 * ii:128 * (ii + 1)],
                                _tr(qh[:, 128 * p:128 * (p + 1)]),
                                _tr(self.ident))
                        nc.vector.tensor_copy(
                            out=_mm(dests[p][:, 512 * g:512 * (g + 1)]),
                            in_=tps.bitcast(F32R) if F32R_MM else tps)
            else:
                vh4 = self.vh.rearrange("p i (h e) -> p i h e", h=H_LOC)
                ps_h = ps.rearrange("p (h e) -> p h e", h=H_LOC)
                nc.vector.tensor_scalar_mul(
                    _mm(vh4[:, i, :, 0:D_HEAD]),
                    ps_h.bitcast(F32R) if F32R_MM else ps_h, r_i)

    # ------------- attention -------------

    def attention(self, st_ps, ac_ps, pt_pool, rs_pool, rb_pool, tb_pool,
                  maskT_sb, padb_sb):
        nc = self.nc
        vh4 = self.vh.rearrange("p i (h e) -> p i h e", h=H_LOC)
        for J in range(NJ):
            accs = [ac_ps.tile([128, 512], F32, tag=f"acc{h}", name=f"acc{h}")
                    for h in range(H_LOC)]
            n_i = 4 * J + 4
            for i in range(n_i):
                start = max(0, 128 * (i - 4 * J))
                for p in range(2):
                    for hh in range(2):
                        h = 2 * p + hh
                        stt = st_ps.tile([128, 512], F32, tag="st")
                        nc.tensor.matmul(
                            stt[:, start:512],
                            _mm(self.khT[p][64 * hh:64 * (hh + 1),
                                            128 * i:128 * (i + 1)]),
                            _mm(self.qhT[p][64 * hh:64 * (hh + 1),
                                            512 * J + start:512 * (J + 1)]),
                            start=True, stop=True,
                            tile_position=(64 * hh, 0))
                        if USE_TANH:
                            nc.scalar.activation(
                                out=stt[:, start:512], in_=stt[:, start:512],
                                func=AF.Tanh, scale=1.0 / (8.0 * GROK))
                            exp_scale = GROK
                        else:
                            exp_scale = 1.0 / 8.0
                        if i >= 4 * J:
                            nc.vector.tensor_add(
                                stt[:, start:start + 128],
                                stt[:, start:start + 128], maskT_sb)
                        pt = pt_pool.tile([128, 512], F32, tag="pt")
                        nc.scalar.activation(
                            out=_mm(pt[:, start:512]), in_=stt[:, start:512],
                            func=AF.Exp, bias=padb_sb[:, i:i + 1],
                            scale=exp_scale)
                        m_h = 65 + 32 * (h % 2)
                        nc.tensor.matmul(
                            accs[h][0:m_h, start:512],
                            _mm(vh4[:, i, h, 0:m_h]),
                            _mm(pt[:, start:512]),
                            start=(i == 0), stop=(i == n_i - 1))
            rs = rs_pool.tile([128, 1024], F32, tag="rs")
            for h in range(H_LOC):
                pp = 64 + 32 * (h % 2)
                fo = 512 * (h // 2)
                nc.vector.tensor_copy(out=rs[pp:pp + 1, fo:fo + 512],
                                      in_=accs[h][pp:pp + 1, :])
            band = slice(64, 97)
            y = rs_pool.tile([128, 1024], F32, tag="ry")
            u = rs_pool.tile([128, 1024], F32, tag="ru")
            nc.vector.tensor_tensor(
                y.bitcast(U32)[band, :], self.recip_c[band, :],
                rs.bitcast(U32)[band, :], op=ALU.subtract)
            for _ in range(2):
                nc.vector.tensor_mul(u[band, :], rs[band, :], y[band, :])
                nc.vector.tensor_scalar(u[band, :], u[band, :], -1.0, 2.0,
                                        op0=ALU.mult, op1=ALU.add)
                nc.vector.tensor_mul(y[band, :], y[band, :], u[band, :])
            for p in range(2):
                for hh in range(2):
                    h = 2 * p + hh
                    pp = 64 + 32 * (h % 2)
                    fo = 512 * (h // 2)
                    nc.sync.dma_start(
                        out=self.rb_dr[h, 512 * J:512 * (J + 1)][None, :],
                        in_=y[pp:pp + 1, fo:fo + 512])
                    rb = rb_pool.tile([64, 512], F32, tag="rb")
                    nc.sync.dma_start(
                        out=rb,
                        in_=self.rb_dr[h, 512 * J:512 * (J + 1)]
                        [None, :].to_broadcast([64, 512]))
                    if hh == 0:
                        nc.vector.tensor_mul(
                            _mm(self.outT[p][0:64, 512 * J:512 * (J + 1)]),
                            accs[h][0:64, :], rb)
                    else:
                        tb = tb_pool.tile([64, 512], F32, tag="tb")
                        nc.vector.tensor_mul(_mm(tb), accs[h][0:64, :], rb)
                        nc.sync.dma_start(
                            out=_mm(self.outT[p][64:128,
                                                 512 * J:512 * (J + 1)]),
                            in_=_mm(tb))

    # ------------- o_proj -------------

    def oproj(self, wo_pool, of_pool, op_ps):
        nc = self.nc
        wo_sb = wo_pool.tile([128, 2, D_MODEL], F32, tag="wo_sb")
        nc.sync.dma_start(
            out=_mm(wo_sb),
            in_=_mm(self.d["wo"].rearrange("(c p) n -> p c n", p=128)))
        for i in range(TC):
            of = of_pool.tile([128, D_MODEL], F32, tag="of")
            for half in range(2):
                ps = op_ps.tile([128, 512], F32, tag="op")
                for c in range(2):
                    nc.tensor.matmul(
                        ps,
                        _mm(self.outT[c][:, 128 * i:128 * (i + 1)]),
                        _mm(wo_sb[:, c, 512 * half:512 * (half + 1)]),
                        start=(c == 0), stop=(c == 1))
                nc.scalar.copy(out=of[:, 512 * half:512 * (half + 1)], in_=ps)
            nc.sync.dma_start(
                out=self.out_dram[128 * i:128 * (i + 1), :], in_=of)


# ------------- host side -------------

def host_prepare(inputs):
    q = np.asarray(inputs["query_sequences"], dtype=np.float32)
    k = np.asarray(inputs["key_sequences"], dtype=np.float32)
    v = np.asarray(inputs["value_sequences"], dtype=np.float32)
    mask = np.asarray(inputs["attention_mask"])
    Wq = np.asarray(inputs["Wq"], dtype=np.float32)
    Wk = np.asarray(inputs["Wk"], dtype=np.float32)
    Wv = np.asarray(inputs["Wv"], dtype=np.float32)
    Wo = np.asarray(inputs["Wo"], dtype=np.float32)
    g = np.asarray(inputs["ln_g"], dtype=np.float32)
    b = np.asarray(inputs["ln_b"], dtype=np.float32)
    assert np.allclose(b, 0.0), "nonzero ln_b not supported by this kernel"

    Wqg = g[:, None] * Wq
    Wkg = g[:, None] * Wk
    Wvg = g[:, None] * Wv

    perm = np.concatenate([np.arange(0, D_HEAD, 2), np.arange(1, D_HEAD, 2)])
    permH = np.concatenate([h * D_HEAD + perm for h in range(N_HEADS)])
    Wqp = Wqg[:, permH]
    Wkp = Wkg[:, permH]

    invf = 1.0 / (ROPE_BASE ** (np.arange(0, D_HEAD, 2, dtype=np.float64)
                                / D_HEAD))
    ang = np.arange(T, dtype=np.float64)[:, None] * invf[None, :]
    cos_h = np.cos(ang)
    sin_h = np.sin(ang)
    cosA1 = np.concatenate([cos_h, cos_h], axis=1)
    sinA1 = np.concatenate([-sin_h, sin_h], axis=1)
    cosA = np.tile(cosA1, (1, H_LOC)).astype(np.float32)
    sinA = np.tile(sinA1, (1, H_LOC)).astype(np.float32)

    maskT = np.where(
        np.arange(128)[:, None] > np.arange(128)[None, :], NEG, 0.0
    ).astype(np.float32)

    in_maps = []
    for c in range(N_CORES):
        bi = c // 4
        gidx = c % 4
        hs = slice(DH * gidx, DH * (gidx + 1))
        wq_c = np.ascontiguousarray(Wqp[:, hs])
        wk_c = np.ascontiguousarray(Wkp[:, hs])
        wv_c = np.ascontiguousarray(Wvg[:, hs])
        wo_c = np.ascontiguousarray(Wo[hs, :])
        padbias = np.where(mask[bi], NEG, 0.0).astype(np.float32)
        vconst = np.zeros((128, TC, H_LOC, 36), dtype=np.float32)
        for h in range(H_LOC):
            vconst[:, :, h, 32 * (h % 2)] = 1.0
        vconst = vconst.reshape(128, TC * H_LOC * 36)
        in_maps.append({
            "xq": np.ascontiguousarray(q[bi]),
            "xk": np.ascontiguousarray(k[bi]),
            "xv": np.ascontiguousarray(v[bi]),
            "wq": wq_c, "wk": wk_c, "wv": wv_c, "wo": wo_c,
            "cosA": cosA, "sinA": sinA,
            "negcq_q": -wq_c.sum(axis=0, keepdims=True),
            "negcq_k": -wk_c.sum(axis=0, keepdims=True),
            "negcq_v": -wv_c.sum(axis=0, keepdims=True),
            "padbias": padbias,
            "maskT": maskT,
            "vconst": vconst,
        })
    return in_maps


_CACHE = {}


def build_nc():
    if "nc" not in _CACHE:
        _CACHE["nc"] = KB().build()
    return _CACHE["nc"]


def kernel(**inputs):
    nc = build_nc()
    in_maps = host_prepare(inputs)
    res = run_bass_kernel_spmd(nc, in_maps, list(range(N_CORES)))
    outs = [np.asarray(res.results[c]["out"]) for c in range(N_CORES)]
    full = np.empty((2, T, D_MODEL), dtype=np.float32)
    for bi in range(2):
        acc = np.zeros((T, D_MODEL), dtype=np.float64)
        for gidx in range(4):
            acc += outs[bi * 4 + gidx]
        full[bi] = acc.astype(np.float32)
    return full
